# revision 1
# baseline (speedup 1.0000x reference)
"""Contrastive loss (InfoNCE-style) on 8 Trainium2 NeuronCores.

Reference math (B=8192, D=128, temp=0.07):
    sim = (emb @ emb.T) / temp, diag masked to -1e9
    log_probs = log_softmax(sim, axis=1)
    row_mean_i = mean over positives (same label, j != i) of log_probs[i, :]
    loss = -sum(row_mean_i) / count(rows with >=1 positive)

Decomposition used here:
    log_probs[i, j] = sim[i, j] - lse_i,   lse_i = log(sum_{j!=i} exp(sim[i, j]))
    pos_sum_i  = q_i - pc_i * lse_i, where q_i = sum_{j pos} sim[i, j] (exact,
                 computed on host in f64 via class-summed embeddings) and
                 pc_i = (# rows with same label) - 1 (host, exact integer math)
    => the ONLY O(B^2) quantity is esum_i = sum_{j!=i} exp(sim[i, j]).

Device kernel (per core c, SPMD-uniform via column rotation):
    input embT_c = roll(emb.T, -1024*c, axis=1)  [128, 8192] f32
      -> core c's own 1024 rows are local columns 0..1023, and row-tile t's
         diagonal element sits at local column 128*t + p (p = partition).
    for each of 8 row-tiles [128 rows]:
      16 matmuls (fp32r, N=512) -> PSUM quarters [128, 2048]
      additive diag mask (-30000) on the [128,128] diagonal block (quarter 0)
      scalar-engine activation Exp(in * 1/temp) with accum_out -> per-quarter
      row sums; exp never leaves SBUF scratch.
    output esums [128, 8] (partition p, row-tile t).

Host: lse = log(esum); row_mean = q/pc - lse (where pc>0); reduce to scalar.
"""

import os

import numpy as np

import concourse.bass as bass
import concourse.mybir as mybir
import concourse.tile as tile
from concourse.tile import add_dep_helper
from concourse.bass_utils import run_bass_kernel_spmd

TEMP = 0.07
B = 8192
D = 128
NCORES = 8
RPC = B // NCORES        # 1024 rows per core
NT = RPC // 128          # 8 row-tiles of 128 rows per core
NCH = B // 512           # 16 column chunks of 512
MASK_RAW = -30000.0      # added in raw-dot space; exp((x+MASK)/temp) == 0.0

_CACHE = {}

# test.py introspection: last BassKernelResults from run_bass_kernel_spmd.
last_results = None


def _build_bass():
    f32 = mybir.dt.float32
    f32r = mybir.dt.float32r
    nc = bass.Bass("TRN2", target_bir_lowering=False, debug=False,
                   num_devices=NCORES)
    # blob0: ident (cols 0:128) | mask strip (128:1152) | emb cols 0:2048
    # (1152:3200). blobR: emb cols 2048:8192. One DMA each keeps the DMA-queue
    # semaphore count low enough for walrus's per-instruction sync limits.
    blob0 = nc.dram_tensor("blob0", [128, 3200], f32r, kind="ExternalInput")
    blob1 = nc.dram_tensor("blob1", [128, 6144], f32r, kind="ExternalInput")
    esums = nc.dram_tensor("esums", [128, NT], f32, kind="ExternalOutput")

    with tile.TileContext(nc) as tc:
        with (
            tc.tile_pool(name="big", bufs=1) as big,
            tc.tile_pool(name="psum", bufs=2, space="PSUM") as psum,
            tc.tile_pool(name="scratch", bufs=32) as scratch,
            tc.tile_pool(name="small", bufs=1) as small,
        ):
            blob0_s = big.tile([128, 3200], f32r)
            nc.sync.dma_start(out=blob0_s[:, :], in_=blob0.ap()[:, :])
            in_dma0 = nc.cur_bb.bb.instructions[-1]
            embR = big.tile([128, 6144], f32r)
            nc.sync.dma_start(out=embR[:, :], in_=blob1.ap()[:, :])
            in_dma1 = nc.cur_bb.bb.instructions[-1]
            # manual drains observing each input queue on the SP proc, so the
            # wait-limited kernel-tail drain doesn't need those semaphores
            for dep in (in_dma0, in_dma1):
                nc.sync.drain()
                add_dep_helper(nc.cur_bb.bb.instructions[-1], dep, sync=True,
                               reason="observe input DMA queue on SP")
            ident_s = blob0_s[:, 0:128]
            mstrip_s = blob0_s[:, 128:1152]
            emb0 = blob0_s[:, 1152:3200]

            esum_all = small.tile([128, NT * 4], f32)
            esums_s = small.tile([128, NT], f32)

            # prefetch dummies: a discarded LDWEIGHTS per input DMA, so the
            # PE observes every DMA semaphore here and real matmuls never
            # need to carry more than one sync wait (walrus limit); real
            # matmuls reload their own weights, so the garbage load is inert
            bf16 = mybir.dt.bfloat16
            nc.tensor.ldweights(blob0_s[:, 0:1].bitcast(bf16))
            nc.tensor.ldweights(embR[:, 0:1].bitcast(bf16))

            for t in range(NT):
                lhs = emb0[:, t * 128:(t + 1) * 128]
                kd, od = t // 4, (t % 4) * 128   # diag chunk within quarter 0
                for q in range(4):
                    qi = t * 4 + q
                    ps = psum.tile([128, 2048], f32, tag="ps")
                    carrier = None
                    if qi >= 2:
                        # discarded LDWEIGHTS reading the 2-quarters-ago accum
                        # slice: carries the ACT wait so the slot-reuse matmul
                        # below carries only its PE wait
                        nc.tensor.ldweights(
                            esum_all[:, qi - 2:qi - 1].bitcast(bf16))
                        carrier = nc.cur_bb.bb.instructions[-1]
                    for k in range(4):
                        n = 4 * q + k
                        is_diag = (q == 0 and k == kd)
                        nc.tensor.matmul(
                            ps[:, k * 512:(k + 1) * 512],
                            lhs,
                            (emb0[:, n * 512:(n + 1) * 512] if n < 4 else
                             embR[:, (n - 4) * 512:(n - 3) * 512]),
                            start=True, stop=not is_diag,
                        )
                        if carrier is not None:
                            add_dep_helper(nc.cur_bb.bb.instructions[-1],
                                           carrier, sync=False,
                                           reason="wait-carrier order")
                            carrier = None
                        if is_diag:
                            # accumulate -1e4 onto the diagonal entries:
                            # out[m, n] += sum_k I[k, m] * mstrip[k, n]
                            nc.tensor.matmul(
                                ps[:, k * 512:(k + 1) * 512],
                                ident_s,
                                mstrip_s[:, 512 - od:1024 - od],
                                start=False, stop=True,
                            )
                        last_mm = nc.cur_bb.bb.instructions[-1]
                    scr = scratch.tile([128, 2048], mybir.dt.bfloat16)
                    nc.scalar.activation(
                        scr[:, :], ps[:, :],
                        mybir.ActivationFunctionType.Exp,
                        scale=1.0 / TEMP,
                        accum_out=esum_all[:, t * 4 + q: t * 4 + q + 1],
                    )

            # final [128, 4] -> [128, 1] sums per row-tile on the scalar
            # engine (keeps the vector engine out of the program: fewer
            # semaphores on walrus's wait-limited kernel-tail drain)
            junk = small.tile([128, 4 * NT], f32)
            for t in range(NT):
                nc.scalar.activation(
                    junk[:, t * 4:(t + 1) * 4],
                    esum_all[:, t * 4:(t + 1) * 4],
                    mybir.ActivationFunctionType.Copy,
                    accum_out=esums_s[:, t:t + 1],
                )
            last_act = nc.cur_bb.bb.instructions[-1]
            # one manual drain per outstanding proc, each carrying a single
            # wait, so the auto-generated kernel-tail drain (which tolerates
            # almost no sync waits) has nothing left to wait for
            nc.sync.drain()
            add_dep_helper(nc.cur_bb.bb.instructions[-1], last_mm, sync=True,
                           reason="observe PE on SP")
            nc.sync.drain()
            add_dep_helper(nc.cur_bb.bb.instructions[-1], last_act, sync=True,
                           reason="observe ACT on SP")
            nc.sync.dma_start(out=esums.ap()[:, :], in_=esums_s[:, :])
            out_dma = nc.cur_bb.bb.instructions[-1]
            nc.sync.drain()
            add_dep_helper(nc.cur_bb.bb.instructions[-1], out_dma, sync=True,
                           reason="observe out DMA queue on SP")
    return nc


def _get_nc():
    if "nc" not in _CACHE:
        _CACHE["nc"] = _build_bass()
    return _CACHE["nc"]


def _host_inputs(emb):
    """Per-core in_maps: rotated emb.T plus the diagonal mask tile."""
    embT = np.ascontiguousarray(emb.T.astype(np.float32, copy=False))  # [128, B]
    qidx = np.arange(1024)[None, :]
    pidx = np.arange(128)[:, None]
    maskc = np.concatenate([
        np.eye(128, dtype=np.float32),
        np.where(qidx == pidx + 512, MASK_RAW, 0.0).astype(np.float32),
    ], axis=1)
    in_maps = []
    for c in range(NCORES):
        s = RPC * c
        rolled = np.concatenate([embT[:, s:], embT[:, :s]], axis=1)
        in_maps.append({
            "blob0": np.ascontiguousarray(
                np.concatenate([maskc, rolled[:, 0:2048]], axis=1)),
            "blob1": np.ascontiguousarray(rolled[:, 2048:]),
        })
    return in_maps


def kernel(embeddings, labels):
    global last_results
    emb = np.asarray(embeddings, dtype=np.float32)
    labels = np.asarray(labels).astype(np.int64)
    assert emb.shape == (B, D) and labels.shape == (B,)

    nc = _get_nc()
    in_maps = _host_inputs(emb)
    res = run_bass_kernel_spmd(nc, in_maps, core_ids=list(range(NCORES)))
    last_results = res

    # esums[p, t] on core c -> global row 1024*c + 128*t + p
    esum = np.concatenate(
        [np.asarray(res.results[c]["esums"]).T.reshape(-1) for c in range(NCORES)]
    ).astype(np.float64)

    emb64 = emb.astype(np.float64)
    nclass = int(labels.max()) + 1
    cnt = np.bincount(labels, minlength=nclass)
    pc = cnt[labels] - 1                      # positives per row (excl. self)
    G = np.zeros((nclass, D), dtype=np.float64)
    np.add.at(G, labels, emb64)
    # q_i = sum over positives j (same label, j != i) of sim[i, j]
    q = (np.einsum("ij,ij->i", emb64, G[labels])
         - np.einsum("ij,ij->i", emb64, emb64)) / TEMP

    lse = np.log(esum)
    has = pc > 0
    row_mean = np.where(has, q / np.maximum(pc, 1) - lse, 0.0)
    loss = -row_mean.sum() / max(int(has.sum()), 1)
    return np.float32(loss)



# revision 8
# speedup vs baseline: 9.0310x; 9.0310x over previous
"""Contrastive loss (InfoNCE-style) on 8 Trainium2 NeuronCores.

Reference math (B=8192, D=128, temp=0.07):
    sim = (emb @ emb.T) / temp, diag masked to -1e9
    log_probs = log_softmax(sim, axis=1)
    row_mean_i = mean over positives (same label, j != i) of log_probs[i, :]
    loss = -sum(row_mean_i) / count(rows with >=1 positive)

Decomposition (only the O(B^2) esum runs on device):
    log_probs[i, j] = sim[i, j] - lse_i,  lse_i = log(sum_{j!=i} exp(sim[i, j]))
    pos_sum_i = q_i - pc_i * lse_i with q_i, pc_i exact on host (f64).

Device kernel (SPMD-uniform, no per-core program divergence):
    input  eshard [128, 1024] bf16 = this core's 1024 columns of emb.T
    - AllGather the 8 shards HBM->HBM -> gath [1024, 1024] (block c =
      core c's [128, 1024] shard), then 8 DMAs -> SBUF embT [128, 8192].
    - diag pass: sq = Square(eshard) f32; per row-tile t a [128,1]-moving
      matmul with a ones vector column-sums sq -> ||e_row||^2 in PSUM;
      Exp(x/temp) -> out cols 32:40. This reproduces the in-matmul diag
      term exp(sim_ii/temp) to ~fp32 rounding, so the host can subtract
      it exactly - no diagonal masking (and hence no column rotation or
      per-core mask data) is needed on device.
    - main loop: 8 row-tiles x 4 PSUM quarters x 4 matmuls (bf16, N=512)
      lhs = own shard tile, rhs = gathered embT chunk; scalar-engine
      Exp(x/temp) with f32 accum_out -> esum quarters, out cols 0:32.
    output outall [128, 40] f32: cols 0:32 esum quarters (p, 4t+q),
    cols 32:40 exp(sim_ii/temp) (p, t).

Host: esum_i = sum_q quarters - diagexp_i; lse = log(esum); row means and
the final scalar reduction in f64 exactly as the reference.

bf16 embeddings perturb off-diag sim by ~2.5e-4 (abs), i.e. lse by <1e-3:
far inside the 2e-2 gate. The diag term exp(1/temp) ~ 1.6e6 dwarfs the
true esum ~ 1.8e4, so it is cancelled with the device-computed value
(same bf16 inputs, same ACT exp) rather than a host-side exp.
"""

import numpy as np

import jax
from jax.sharding import Mesh, PartitionSpec
from jax.experimental.shard_map import shard_map

import concourse.bass as bass
import concourse.mybir as mybir
import concourse.tile as tile
from concourse.tile import add_dep_helper

TEMP = 0.07
B = 8192
D = 128
NCORES = 8
RPC = B // NCORES        # 1024 rows per core
NT = RPC // 128          # 8 row-tiles of 128 rows per core
NQ = 4                   # 4 PSUM quarters of 2048 columns
OUTW = NT * NQ + NT      # 32 esum quarters + 8 diag exps

_CACHE = {}

# test.py introspection hook (unused by this runner; kept for compat).
last_results = None


def _build_bass():
    f32 = mybir.dt.float32
    bf16 = mybir.dt.bfloat16
    nc = bass.Bass("TRN2", target_bir_lowering=False, debug=False,
                   num_devices=NCORES)
    eshard = nc.dram_tensor("eshard", [128, RPC], bf16, kind="ExternalInput")
    outall = nc.dram_tensor("outall", [128, OUTW], f32, kind="ExternalOutput")

    with tile.TileContext(nc) as tc:
        with (
            tc.tile_pool(name="big", bufs=1) as big,
            tc.tile_pool(name="psum", bufs=2, space="PSUM") as psum,
            tc.tile_pool(name="scratch", bufs=32) as scratch,
            tc.tile_pool(name="small", bufs=1) as small,
            tc.tile_pool(name="dram", bufs=1, space="DRAM") as dram,
        ):
            es = big.tile([128, RPC], bf16)
            nc.sync.dma_start(out=es[:, :], in_=eshard.ap()[:, :])
            es_dma = nc.cur_bb.bb.instructions[-1]

            ones = small.tile([128, 1], f32)
            nc.gpsimd.memset(ones[:, :], 1.0)

            # HBM bounce -> AllGather: gath block c = core c's shard.
            inb = dram.tile([128, RPC], bf16)
            gath = dram.tile([NCORES * 128, RPC], bf16)
            nc.gpsimd.dma_start(inb[:, :], eshard.ap()[:, :])
            bounce_dma = nc.cur_bb.bb.instructions[-1]
            nc.gpsimd.collective_compute(
                "AllGather", mybir.AluOpType.bypass,
                replica_groups=[list(range(NCORES))],
                ins=[inb.opt()], outs=[gath.opt()],
            )
            cc_inst = nc.cur_bb.bb.instructions[-1]

            # one 3D-AP DMA moves all 8 gathered blocks into SBUF
            # (embT[p, c*1024+j] = gath[c*128+p, j]); a single DMA keeps the
            # SP queue count low enough for walrus's wait-limited kernel-tail
            # drain (8 separate DMAs reproducibly overflow it)
            embT = big.tile([128, B], bf16)
            nc.sync.dma_start(
                out=embT[:, :].rearrange("p (c j) -> p c j", c=NCORES),
                in_=gath[:, :].rearrange("(c p) j -> p c j", p=128))
            gather_dma = nc.cur_bb.bb.instructions[-1]

            # manual single-wait drains per input queue / collective, so the
            # wait-limited kernel-tail drain has nothing left to observe
            for dep in (es_dma, bounce_dma, cc_inst, gather_dma):
                nc.sync.drain()
                add_dep_helper(nc.cur_bb.bb.instructions[-1], dep, sync=True,
                               reason="observe producer on SP")

            # prefetch dummies: a discarded LDWEIGHTS per semaphore the PE
            # must observe, so real matmuls carry at most one sync wait
            nc.tensor.ldweights(es[:, 0:1])
            nc.tensor.ldweights(embT[:, 0:1])
            nc.tensor.ldweights(ones[:, :].bitcast(bf16))

            esum_all = small.tile([128, OUTW], f32)

            # diag pass: sq = e^2 (f32), column sums via ones-moving matmuls,
            # exp(x/temp) -> esum_all[:, 32:40]
            sq = big.tile([128, RPC], f32)
            nc.scalar.activation(sq[:, :], es[:, :],
                                 mybir.ActivationFunctionType.Square)
            nc.tensor.ldweights(sq[:, 0:1].bitcast(bf16))
            pd = psum.tile([128, 2048], f32, tag="ps")
            for t in range(NT):
                nc.tensor.matmul(
                    pd[:, t:t + 1],
                    sq[:, t * 128:(t + 1) * 128],
                    ones[:, :],
                    start=True, stop=True,
                )
            nc.scalar.activation(
                esum_all[:, NT * NQ:OUTW], pd[:, 0:NT],
                mybir.ActivationFunctionType.Exp,
                scale=1.0 / TEMP,
            )

            # main loop: esum quarters
            for t in range(NT):
                lhs = es[:, t * 128:(t + 1) * 128]
                for q in range(NQ):
                    qi = t * NQ + q
                    ps = psum.tile([128, 2048], f32, tag="ps")
                    # discarded LDWEIGHTS reading the ACT output that retired
                    # this PSUM slot: carries the ACT wait so the slot-reuse
                    # matmul below carries only its PE wait
                    if qi == 1:
                        nc.tensor.ldweights(
                            esum_all[:, NT * NQ:NT * NQ + 1].bitcast(bf16))
                        carrier = nc.cur_bb.bb.instructions[-1]
                    elif qi >= 2:
                        nc.tensor.ldweights(
                            esum_all[:, qi - 2:qi - 1].bitcast(bf16))
                        carrier = nc.cur_bb.bb.instructions[-1]
                    else:
                        carrier = None
                    for k in range(4):
                        n = NQ * q + k
                        nc.tensor.matmul(
                            ps[:, k * 512:(k + 1) * 512],
                            lhs,
                            embT[:, n * 512:(n + 1) * 512],
                            start=True, stop=True,
                        )
                        if carrier is not None:
                            add_dep_helper(nc.cur_bb.bb.instructions[-1],
                                           carrier, sync=False,
                                           reason="wait-carrier order")
                            carrier = None
                        last_mm = nc.cur_bb.bb.instructions[-1]
                    scr = scratch.tile([128, 2048], mybir.dt.bfloat16)
                    nc.scalar.activation(
                        scr[:, :], ps[:, :],
                        mybir.ActivationFunctionType.Exp,
                        scale=1.0 / TEMP,
                        accum_out=esum_all[:, qi:qi + 1],
                    )
            # single-writer funnel: the out DMA may carry only a few sync
            # waits, but esum_all has 33 ACT writers. One ACT Copy (in-order
            # wrt all prior ACT writes, so no extra semaphores) gives the
            # DMA exactly one producer.
            out_s = small.tile([128, OUTW], f32)
            nc.scalar.activation(out_s[:, :], esum_all[:, :],
                                 mybir.ActivationFunctionType.Copy)
            last_act = nc.cur_bb.bb.instructions[-1]

            nc.sync.drain()
            add_dep_helper(nc.cur_bb.bb.instructions[-1], last_mm, sync=True,
                           reason="observe PE on SP")
            nc.sync.drain()
            add_dep_helper(nc.cur_bb.bb.instructions[-1], last_act, sync=True,
                           reason="observe ACT on SP")
            # issue the out DMA from the ACT engine: it directly follows the
            # ACT Copy in the same stream, so it needs no sync waits at all
            nc.scalar.dma_start(out=outall.ap()[:, :], in_=out_s[:, :])
            out_dma = nc.cur_bb.bb.instructions[-1]
            nc.sync.drain()
            add_dep_helper(nc.cur_bb.bb.instructions[-1], out_dma, sync=True,
                           reason="observe out DMA queue on SP")
    return nc


def _get_runner():
    """Build the Bass module once and return a cached jitted SPMD callable.

    Replicates concourse.bass2jax.run_bass_via_pjrt but keeps the jitted
    function across calls: re-tracing + re-lowering per call costs hundreds
    of ms through the axon tunnel, far more than this kernel's transfers.
    """
    if "runner" in _CACHE:
        return _CACHE["runner"]

    from concourse.bass2jax import (
        _bass_exec_p, install_neuronx_cc_hook, partition_id_tensor,
    )

    nc = _build_bass()
    install_neuronx_cc_hook()

    partition_name = (nc.partition_id_tensor.name
                      if nc.partition_id_tensor else None)
    in_names, out_names, out_avals, zero_shapes = [], [], [], []
    for alloc in nc.m.functions[0].allocations:
        if not isinstance(alloc, mybir.MemoryLocationSet):
            continue
        name = alloc.memorylocations[0].name
        if alloc.kind == "ExternalInput":
            if name != partition_name:
                in_names.append(name)
        elif alloc.kind == "ExternalOutput":
            shape = tuple(alloc.tensor_shape)
            dtype = mybir.dt.np(alloc.dtype)
            out_names.append(name)
            out_avals.append(jax.core.ShapedArray(shape, dtype))
            zero_shapes.append((shape, dtype))
    n_params = len(in_names)
    n_outs = len(out_names)
    in_names_all = list(in_names) + list(out_names)
    if partition_name is not None:
        in_names_all.append(partition_name)
    donate = tuple(range(n_params, n_params + n_outs))

    def _body(*args):
        operands = list(args)
        if partition_name is not None:
            operands.append(partition_id_tensor())
        outs = _bass_exec_p.bind(
            *operands,
            out_avals=tuple(out_avals),
            in_names=tuple(in_names_all),
            out_names=tuple(out_names),
            lowering_input_output_aliases=(),
            sim_require_finite=True,
            sim_require_nnan=True,
            nc=nc,
        )
        return tuple(outs)

    devices = jax.devices()[:NCORES]
    assert len(devices) == NCORES, (
        f"need {NCORES} devices, found {len(jax.devices())}")
    mesh = Mesh(np.asarray(devices), ("core",))
    sharded = jax.jit(
        shard_map(_body, mesh=mesh,
                  in_specs=(PartitionSpec("core"),) * (n_params + n_outs),
                  out_specs=(PartitionSpec("core"),) * n_outs,
                  check_rep=False),
        donate_argnums=donate,
        keep_unused=True,
    )
    _CACHE["runner"] = (sharded, in_names, zero_shapes)
    return _CACHE["runner"]


def kernel(embeddings, labels):
    emb = np.asarray(embeddings, dtype=np.float32)
    labels = np.asarray(labels).astype(np.int64)
    assert emb.shape == (B, D) and labels.shape == (B,)

    sharded, in_names, zero_shapes = _get_runner()

    bf16 = jax.numpy.bfloat16
    embT = np.ascontiguousarray(emb.T).astype(bf16)          # [128, B]
    # shard c = embT[:, 1024c:1024(c+1)], stacked on axis 0 for shard_map
    eshard_cat = np.ascontiguousarray(
        embT.reshape(128, NCORES, RPC).transpose(1, 0, 2)
    ).reshape(NCORES * 128, RPC)
    zeros = [np.zeros((NCORES * s[0], *s[1:]), dt) for s, dt in zero_shapes]

    out_arrs = sharded(eshard_cat, *zeros)
    oa = np.asarray(out_arrs[0]).reshape(NCORES, 128, OUTW).astype(np.float64)

    # outall[c, p, 4t+q] -> esum quarters; outall[c, p, 32+t] -> diag exp.
    # local row j = 128t + p, global row = 1024c + j.
    quarters = oa[:, :, :NT * NQ].reshape(NCORES, 128, NT, NQ).sum(axis=3)
    esum = quarters.transpose(0, 2, 1).reshape(-1)           # [B]
    dexp = oa[:, :, NT * NQ:].transpose(0, 2, 1).reshape(-1)  # [B]
    esum_nodiag = esum - dexp
    lse = np.log(esum_nodiag)

    emb64 = emb.astype(np.float64)
    nclass = int(labels.max()) + 1
    cnt = np.bincount(labels, minlength=nclass)
    pc = cnt[labels] - 1                      # positives per row (excl. self)
    G = np.zeros((nclass, D), dtype=np.float64)
    np.add.at(G, labels, emb64)
    # q_i = sum over positives j (same label, j != i) of sim[i, j]
    q = (np.einsum("ij,ij->i", emb64, G[labels])
         - np.einsum("ij,ij->i", emb64, emb64)) / TEMP

    has = pc > 0
    row_mean = np.where(has, q / np.maximum(pc, 1) - lse, 0.0)
    loss = -row_mean.sum() / max(int(has.sum()), 1)
    return np.float32(loss)


# revision 10
# speedup vs baseline: 12.6413x; 1.3998x over previous
"""Contrastive loss (InfoNCE-style) on 8 Trainium2 NeuronCores.

Reference math (B=8192, D=128, temp=0.07):
    sim = (emb @ emb.T) / temp, diag masked to -1e9
    log_probs = log_softmax(sim, axis=1)
    row_mean_i = mean over positives (same label, j != i) of log_probs[i, :]
    loss = -sum(row_mean_i) / count(rows with >=1 positive)

Decomposition (only the O(B^2) esum runs on device):
    log_probs[i, j] = sim[i, j] - lse_i,  lse_i = log(sum_{j!=i} exp(sim[i, j]))
    pos_sum_i = q_i - pc_i * lse_i with q_i, pc_i exact on host (f64).

Device kernel (SPMD-uniform, no per-core program divergence):
    input  eshard [128, 1024] bf16 = this core's 1024 columns of emb.T
    - AllGather the 8 shards HBM->HBM -> gath [1024, 1024] (block c =
      core c's [128, 1024] shard), then 8 DMAs -> SBUF embT [128, 8192].
    - diag pass: sq = Square(eshard) f32; per row-tile t a [128,1]-moving
      matmul with a ones vector column-sums sq -> ||e_row||^2 in PSUM;
      Exp(x/temp) -> out cols 32:40. This reproduces the in-matmul diag
      term exp(sim_ii/temp) to ~fp32 rounding, so the host can subtract
      it exactly - no diagonal masking (and hence no column rotation or
      per-core mask data) is needed on device.
    - main loop: 8 row-tiles x 4 PSUM quarters x 4 matmuls (bf16, N=512)
      lhs = own shard tile, rhs = gathered embT chunk; scalar-engine
      Exp(x/temp) with f32 accum_out -> esum quarters, out cols 0:32.
    output outall [128, 40] f32: cols 0:32 esum quarters (p, 4t+q),
    cols 32:40 exp(sim_ii/temp) (p, t).

Host: esum_i = sum_q quarters - diagexp_i; lse = log(esum); row means and
the final scalar reduction in f64 exactly as the reference.

bf16 embeddings perturb off-diag sim by ~2.5e-4 (abs), i.e. lse by <1e-3:
far inside the 2e-2 gate. The diag term exp(1/temp) ~ 1.6e6 dwarfs the
true esum ~ 1.8e4, so it is cancelled with the device-computed value
(same bf16 inputs, same ACT exp) rather than a host-side exp.
"""

import numpy as np

import jax
from jax.sharding import Mesh, PartitionSpec
from jax.experimental.shard_map import shard_map

import concourse.bass as bass
import concourse.mybir as mybir
import concourse.tile as tile
from concourse.tile import add_dep_helper

TEMP = 0.07
B = 8192
D = 128
NCORES = 8
RPC = B // NCORES        # 1024 rows per core
NT = RPC // 128          # 8 row-tiles of 128 rows per core
NQ = 4                   # 4 PSUM quarters of 2048 columns
OUTW = NT * NQ + NT      # 32 esum quarters + 8 diag exps

_CACHE = {}

# test.py introspection hook (unused by this runner; kept for compat).
last_results = None


def _build_bass():
    f32 = mybir.dt.float32
    bf16 = mybir.dt.bfloat16
    nc = bass.Bass("TRN2", target_bir_lowering=False, debug=False,
                   num_devices=NCORES)
    eshard = nc.dram_tensor("eshard", [128, RPC], bf16, kind="ExternalInput")
    outall = nc.dram_tensor("outall", [128, OUTW], f32, kind="ExternalOutput")

    with tile.TileContext(nc) as tc:
        with (
            tc.tile_pool(name="big", bufs=1) as big,
            tc.tile_pool(name="psum", bufs=2, space="PSUM") as psum,
            tc.tile_pool(name="scratch", bufs=32) as scratch,
            tc.tile_pool(name="small", bufs=1) as small,
            tc.tile_pool(name="dram", bufs=1, space="DRAM") as dram,
        ):
            es = big.tile([128, RPC], bf16)
            nc.sync.dma_start(out=es[:, :], in_=eshard.ap()[:, :])
            es_dma = nc.cur_bb.bb.instructions[-1]

            ones = small.tile([128, 1], f32)
            nc.gpsimd.memset(ones[:, :], 1.0)

            # HBM bounce -> AllGather: gath block c = core c's shard.
            inb = dram.tile([128, RPC], bf16)
            gath = dram.tile([NCORES * 128, RPC], bf16)
            nc.gpsimd.dma_start(inb[:, :], eshard.ap()[:, :])
            bounce_dma = nc.cur_bb.bb.instructions[-1]
            nc.gpsimd.collective_compute(
                "AllGather", mybir.AluOpType.bypass,
                replica_groups=[list(range(NCORES))],
                ins=[inb.opt()], outs=[gath.opt()],
            )
            cc_inst = nc.cur_bb.bb.instructions[-1]

            # one 3D-AP DMA moves all 8 gathered blocks into SBUF
            # (embT[p, c*1024+j] = gath[c*128+p, j]); a single DMA keeps the
            # SP queue count low enough for walrus's wait-limited kernel-tail
            # drain (8 separate DMAs reproducibly overflow it)
            embT = big.tile([128, B], bf16)
            nc.sync.dma_start(
                out=embT[:, :].rearrange("p (c j) -> p c j", c=NCORES),
                in_=gath[:, :].rearrange("(c p) j -> p c j", p=128))
            gather_dma = nc.cur_bb.bb.instructions[-1]

            # manual single-wait drains per input queue / collective, so the
            # wait-limited kernel-tail drain has nothing left to observe
            for dep in (es_dma, bounce_dma, cc_inst, gather_dma):
                nc.sync.drain()
                add_dep_helper(nc.cur_bb.bb.instructions[-1], dep, sync=True,
                               reason="observe producer on SP")

            # prefetch dummies: a discarded LDWEIGHTS per semaphore the PE
            # must observe, so real matmuls carry at most one sync wait
            nc.tensor.ldweights(es[:, 0:1])
            nc.tensor.ldweights(embT[:, 0:1])
            nc.tensor.ldweights(ones[:, :].bitcast(bf16))

            esum_all = small.tile([128, OUTW], f32)

            # diag pass: sq = e^2 (f32), column sums via ones-moving matmuls,
            # exp(x/temp) -> esum_all[:, 32:40]
            sq = big.tile([128, RPC], f32)
            nc.scalar.activation(sq[:, :], es[:, :],
                                 mybir.ActivationFunctionType.Square)
            nc.tensor.ldweights(sq[:, 0:1].bitcast(bf16))
            pd = psum.tile([128, 2048], f32, tag="ps")
            for t in range(NT):
                nc.tensor.matmul(
                    pd[:, t:t + 1],
                    sq[:, t * 128:(t + 1) * 128],
                    ones[:, :],
                    start=True, stop=True,
                )
            nc.scalar.activation(
                esum_all[:, NT * NQ:OUTW], pd[:, 0:NT],
                mybir.ActivationFunctionType.Exp,
                scale=1.0 / TEMP,
            )

            # main loop: esum quarters
            for t in range(NT):
                lhs = es[:, t * 128:(t + 1) * 128]
                for q in range(NQ):
                    qi = t * NQ + q
                    ps = psum.tile([128, 2048], f32, tag="ps")
                    # discarded LDWEIGHTS reading the ACT output that retired
                    # this PSUM slot: carries the ACT wait so the slot-reuse
                    # matmul below carries only its PE wait
                    if qi == 1:
                        nc.tensor.ldweights(
                            esum_all[:, NT * NQ:NT * NQ + 1].bitcast(bf16))
                        carrier = nc.cur_bb.bb.instructions[-1]
                    elif qi >= 2:
                        nc.tensor.ldweights(
                            esum_all[:, qi - 2:qi - 1].bitcast(bf16))
                        carrier = nc.cur_bb.bb.instructions[-1]
                    else:
                        carrier = None
                    for k in range(4):
                        n = NQ * q + k
                        nc.tensor.matmul(
                            ps[:, k * 512:(k + 1) * 512],
                            lhs,
                            embT[:, n * 512:(n + 1) * 512],
                            start=True, stop=True,
                        )
                        if carrier is not None:
                            add_dep_helper(nc.cur_bb.bb.instructions[-1],
                                           carrier, sync=False,
                                           reason="wait-carrier order")
                            carrier = None
                        last_mm = nc.cur_bb.bb.instructions[-1]
                    scr = scratch.tile([128, 2048], mybir.dt.bfloat16)
                    nc.scalar.activation(
                        scr[:, :], ps[:, :],
                        mybir.ActivationFunctionType.Exp,
                        scale=1.0 / TEMP,
                        accum_out=esum_all[:, qi:qi + 1],
                    )
            # single-writer funnel: the out DMA may carry only a few sync
            # waits, but esum_all has 33 ACT writers. One ACT Copy (in-order
            # wrt all prior ACT writes, so no extra semaphores) gives the
            # DMA exactly one producer.
            out_s = small.tile([128, OUTW], f32)
            nc.scalar.activation(out_s[:, :], esum_all[:, :],
                                 mybir.ActivationFunctionType.Copy)
            last_act = nc.cur_bb.bb.instructions[-1]

            nc.sync.drain()
            add_dep_helper(nc.cur_bb.bb.instructions[-1], last_mm, sync=True,
                           reason="observe PE on SP")
            nc.sync.drain()
            add_dep_helper(nc.cur_bb.bb.instructions[-1], last_act, sync=True,
                           reason="observe ACT on SP")
            # issue the out DMA from the ACT engine: it directly follows the
            # ACT Copy in the same stream, so it needs no sync waits at all
            nc.scalar.dma_start(out=outall.ap()[:, :], in_=out_s[:, :])
            out_dma = nc.cur_bb.bb.instructions[-1]
            nc.sync.drain()
            add_dep_helper(nc.cur_bb.bb.instructions[-1], out_dma, sync=True,
                           reason="observe out DMA queue on SP")
    return nc


def _get_runner():
    """Build the Bass module once and return a cached jitted SPMD callable.

    Replicates concourse.bass2jax.run_bass_via_pjrt but keeps the jitted
    function across calls: re-tracing + re-lowering per call costs hundreds
    of ms through the axon tunnel, far more than this kernel's transfers.
    """
    if "runner" in _CACHE:
        return _CACHE["runner"]

    from concourse.bass2jax import (
        _bass_exec_p, install_neuronx_cc_hook, partition_id_tensor,
    )

    nc = _build_bass()
    install_neuronx_cc_hook()

    partition_name = (nc.partition_id_tensor.name
                      if nc.partition_id_tensor else None)
    in_names, out_names, out_avals, zero_shapes = [], [], [], []
    for alloc in nc.m.functions[0].allocations:
        if not isinstance(alloc, mybir.MemoryLocationSet):
            continue
        name = alloc.memorylocations[0].name
        if alloc.kind == "ExternalInput":
            if name != partition_name:
                in_names.append(name)
        elif alloc.kind == "ExternalOutput":
            shape = tuple(alloc.tensor_shape)
            dtype = mybir.dt.np(alloc.dtype)
            out_names.append(name)
            out_avals.append(jax.core.ShapedArray(shape, dtype))
            zero_shapes.append((shape, dtype))
    n_params = len(in_names)
    n_outs = len(out_names)
    in_names_all = list(in_names) + list(out_names)
    if partition_name is not None:
        in_names_all.append(partition_name)
    donate = tuple(range(n_params, n_params + n_outs))

    def _body(*args):
        operands = list(args)
        if partition_name is not None:
            operands.append(partition_id_tensor())
        outs = _bass_exec_p.bind(
            *operands,
            out_avals=tuple(out_avals),
            in_names=tuple(in_names_all),
            out_names=tuple(out_names),
            lowering_input_output_aliases=(),
            sim_require_finite=True,
            sim_require_nnan=True,
            nc=nc,
        )
        return tuple(outs)

    devices = jax.devices()[:NCORES]
    assert len(devices) == NCORES, (
        f"need {NCORES} devices, found {len(jax.devices())}")
    mesh = Mesh(np.asarray(devices), ("core",))
    sharded = jax.jit(
        shard_map(_body, mesh=mesh,
                  in_specs=(PartitionSpec("core"),) * (n_params + n_outs),
                  out_specs=(PartitionSpec("core"),) * n_outs,
                  check_rep=False),
        keep_unused=True,
    )
    # The "output" operands only exist because run_neff-style kernels may
    # rely on pre-zeroed output buffers; this kernel writes every element
    # and the custom call produces fresh result buffers (no aliasing), so
    # park the zeros on device once and never re-upload them.
    from jax.sharding import NamedSharding
    zsh = NamedSharding(mesh, PartitionSpec("core"))
    dev_zeros = [
        jax.device_put(np.zeros((NCORES * s[0], *s[1:]), dt), zsh)
        for s, dt in zero_shapes
    ]
    jax.block_until_ready(dev_zeros)
    _CACHE["runner"] = (sharded, in_names, dev_zeros)
    return _CACHE["runner"]


def kernel(embeddings, labels):
    emb = np.asarray(embeddings, dtype=np.float32)
    labels = np.asarray(labels).astype(np.int64)
    assert emb.shape == (B, D) and labels.shape == (B,)

    sharded, in_names, dev_zeros = _get_runner()

    bf16 = jax.numpy.bfloat16
    # eshard_cat[c*128 + d, j] = bf16(emb[c*1024 + j, d]) in one strided pass
    eshard_cat = (emb.reshape(NCORES, RPC, D).transpose(0, 2, 1)
                  .astype(bf16).reshape(NCORES * 128, RPC))

    # async dispatch: upload + exec + (eager) fetch run while the host
    # computes the exact q/pc terms below
    out_arrs = sharded(eshard_cat, *dev_zeros)

    emb64 = emb.astype(np.float64)
    nclass = int(labels.max()) + 1
    cnt = np.bincount(labels, minlength=nclass)
    pc = cnt[labels] - 1                      # positives per row (excl. self)
    G = np.zeros((nclass, D), dtype=np.float64)
    np.add.at(G, labels, emb64)
    # q_i = sum over positives j (same label, j != i) of sim[i, j]
    q = (np.einsum("ij,ij->i", emb64, G[labels])
         - np.einsum("ij,ij->i", emb64, emb64)) / TEMP
    has = pc > 0

    oa = np.asarray(out_arrs[0]).reshape(NCORES, 128, OUTW).astype(np.float64)
    # outall[c, p, 4t+q] -> esum quarters; outall[c, p, 32+t] -> diag exp.
    # local row j = 128t + p, global row = 1024c + j.
    quarters = oa[:, :, :NT * NQ].reshape(NCORES, 128, NT, NQ).sum(axis=3)
    esum = quarters.transpose(0, 2, 1).reshape(-1)           # [B]
    dexp = oa[:, :, NT * NQ:].transpose(0, 2, 1).reshape(-1)  # [B]
    lse = np.log(esum - dexp)

    row_mean = np.where(has, q / np.maximum(pc, 1) - lse, 0.0)
    loss = -row_mean.sum() / max(int(has.sum()), 1)
    return np.float32(loss)


# revision 12
# speedup vs baseline: 16.6048x; 1.3135x over previous
"""Contrastive loss (InfoNCE-style) on 8 Trainium2 NeuronCores.

Reference math (B=8192, D=128, temp=0.07):
    sim = (emb @ emb.T) / temp, diag masked to -1e9
    log_probs = log_softmax(sim, axis=1)
    row_mean_i = mean over positives (same label, j != i) of log_probs[i, :]
    loss = -sum(row_mean_i) / count(rows with >=1 positive)

Decomposition (only the O(B^2) esum runs on device):
    log_probs[i, j] = sim[i, j] - lse_i,  lse_i = log(sum_{j!=i} exp(sim[i, j]))
    pos_sum_i = q_i - pc_i * lse_i with q_i, pc_i exact on host (f64).

Device kernel (SPMD-uniform, no per-core program divergence):
    input  eshard [128, 1024] bf16 = this core's 1024 columns of emb.T
    - AllGather the 8 shards HBM->HBM -> gath [1024, 1024] (block c =
      core c's [128, 1024] shard), then 8 DMAs -> SBUF embT [128, 8192].
    - diag pass: sq = Square(eshard) f32; per row-tile t a [128,1]-moving
      matmul with a ones vector column-sums sq -> ||e_row||^2 in PSUM;
      Exp(x/temp) -> out cols 32:40. This reproduces the in-matmul diag
      term exp(sim_ii/temp) to ~fp32 rounding, so the host can subtract
      it exactly - no diagonal masking (and hence no column rotation or
      per-core mask data) is needed on device.
    - main loop: 8 row-tiles x 4 PSUM quarters x 4 matmuls (bf16, N=512)
      lhs = own shard tile, rhs = gathered embT chunk; scalar-engine
      Exp(x/temp) with f32 accum_out -> esum quarters, out cols 0:32.
    output outall [128, 40] f32: cols 0:32 esum quarters (p, 4t+q),
    cols 32:40 exp(sim_ii/temp) (p, t).

Host: esum_i = sum_q quarters - diagexp_i; lse = log(esum); row means and
the final scalar reduction in f64 exactly as the reference.

bf16 embeddings perturb off-diag sim by ~2.5e-4 (abs), i.e. lse by <1e-3:
far inside the 2e-2 gate. The diag term exp(1/temp) ~ 1.6e6 dwarfs the
true esum ~ 1.8e4, so it is cancelled with the device-computed value
(same bf16 inputs, same ACT exp) rather than a host-side exp.
"""

import numpy as np

import jax
from jax.sharding import Mesh, PartitionSpec
from jax.experimental.shard_map import shard_map

import concourse.bass as bass
import concourse.mybir as mybir
import concourse.tile as tile
from concourse.tile import add_dep_helper

TEMP = 0.07
B = 8192
D = 128
NCORES = 8
RPC = B // NCORES        # 1024 rows per core
NT = RPC // 128          # 8 row-tiles of 128 rows per core
NQ = 4                   # 4 PSUM quarters of 2048 columns
OUTW = 2 * NT            # 8 esum row-sums + 8 diag exps

_CACHE = {}

# test.py introspection hook (unused by this runner; kept for compat).
last_results = None


def _build_bass():
    f32 = mybir.dt.float32
    bf16 = mybir.dt.bfloat16
    fp8 = mybir.dt.float8e4
    nc = bass.Bass("TRN2", target_bir_lowering=False, debug=False,
                   num_devices=NCORES)
    eshard = nc.dram_tensor("eshard", [128, RPC], fp8, kind="ExternalInput")
    outall = nc.dram_tensor("outall", [128, OUTW], f32, kind="ExternalOutput")

    with tile.TileContext(nc) as tc:
        with (
            tc.tile_pool(name="big", bufs=1) as big,
            tc.tile_pool(name="psum", bufs=2, space="PSUM") as psum,
            tc.tile_pool(name="scratch", bufs=32) as scratch,
            tc.tile_pool(name="small", bufs=1) as small,
            tc.tile_pool(name="dram", bufs=1, space="DRAM") as dram,
        ):
            es = big.tile([128, RPC], fp8)
            nc.sync.dma_start(out=es[:, :], in_=eshard.ap()[:, :])
            es_dma = nc.cur_bb.bb.instructions[-1]

            ones = small.tile([128, 1], f32)
            nc.gpsimd.memset(ones[:, :], 1.0)

            # HBM bounce -> AllGather: gath block c = core c's shard.
            inb = dram.tile([128, RPC], fp8)
            gath = dram.tile([NCORES * 128, RPC], fp8)
            nc.gpsimd.dma_start(inb[:, :], eshard.ap()[:, :])
            bounce_dma = nc.cur_bb.bb.instructions[-1]
            nc.gpsimd.collective_compute(
                "AllGather", mybir.AluOpType.bypass,
                replica_groups=[list(range(NCORES))],
                ins=[inb.opt()], outs=[gath.opt()],
            )
            cc_inst = nc.cur_bb.bb.instructions[-1]

            # one 3D-AP DMA moves all 8 gathered blocks into SBUF
            # (embT[p, c*1024+j] = gath[c*128+p, j]); a single DMA keeps the
            # SP queue count low enough for walrus's wait-limited kernel-tail
            # drain (8 separate DMAs reproducibly overflow it)
            embT = big.tile([128, B], fp8)
            nc.sync.dma_start(
                out=embT[:, :].rearrange("p (c j) -> p c j", c=NCORES),
                in_=gath[:, :].rearrange("(c p) j -> p c j", p=128))
            gather_dma = nc.cur_bb.bb.instructions[-1]

            # manual single-wait drains per input queue / collective, so the
            # wait-limited kernel-tail drain has nothing left to observe
            for dep in (es_dma, bounce_dma, cc_inst, gather_dma):
                nc.sync.drain()
                add_dep_helper(nc.cur_bb.bb.instructions[-1], dep, sync=True,
                               reason="observe producer on SP")

            # prefetch dummies: a discarded LDWEIGHTS per semaphore the PE
            # must observe, so real matmuls carry at most one sync wait
            nc.tensor.ldweights(es[:, 0:2].bitcast(bf16))
            nc.tensor.ldweights(embT[:, 0:2].bitcast(bf16))
            nc.tensor.ldweights(ones[:, :].bitcast(bf16))

            esum_all = small.tile([128, NT * NQ], f32)
            esums_s = small.tile([128, OUTW], f32)

            # diag pass: sq = e^2 (f32), column sums via ones-moving matmuls,
            # exp(x/temp) -> esum_all[:, 32:40]
            sq = big.tile([128, RPC], f32)
            nc.scalar.activation(sq[:, :], es[:, :],
                                 mybir.ActivationFunctionType.Square)
            nc.tensor.ldweights(sq[:, 0:1].bitcast(bf16))
            pd = psum.tile([128, 2048], f32, tag="ps")
            for t in range(NT):
                nc.tensor.matmul(
                    pd[:, t:t + 1],
                    sq[:, t * 128:(t + 1) * 128],
                    ones[:, :],
                    start=True, stop=True,
                )
            nc.scalar.activation(
                esums_s[:, NT:2 * NT], pd[:, 0:NT],
                mybir.ActivationFunctionType.Exp,
                scale=1.0 / TEMP,
            )

            # main loop: esum quarters
            for t in range(NT):
                lhs = es[:, t * 128:(t + 1) * 128]
                for q in range(NQ):
                    qi = t * NQ + q
                    ps = psum.tile([128, 2048], f32, tag="ps")
                    # discarded LDWEIGHTS reading the ACT output that retired
                    # this PSUM slot: carries the ACT wait so the slot-reuse
                    # matmul below carries only its PE wait
                    if qi == 1:
                        nc.tensor.ldweights(
                            esums_s[:, NT:NT + 1].bitcast(bf16))
                        carrier = nc.cur_bb.bb.instructions[-1]
                    elif qi >= 2:
                        nc.tensor.ldweights(
                            esum_all[:, qi - 2:qi - 1].bitcast(bf16))
                        carrier = nc.cur_bb.bb.instructions[-1]
                    else:
                        carrier = None
                    for k in range(4):
                        n = NQ * q + k
                        nc.tensor.matmul(
                            ps[:, k * 512:(k + 1) * 512],
                            lhs,
                            embT[:, n * 512:(n + 1) * 512],
                            start=True, stop=True,
                        )
                        if carrier is not None:
                            add_dep_helper(nc.cur_bb.bb.instructions[-1],
                                           carrier, sync=False,
                                           reason="wait-carrier order")
                            carrier = None
                        last_mm = nc.cur_bb.bb.instructions[-1]
                    scr = scratch.tile([128, 2048], mybir.dt.bfloat16)
                    nc.scalar.activation(
                        scr[:, :], ps[:, :],
                        mybir.ActivationFunctionType.Exp,
                        scale=1.0 / TEMP,
                        accum_out=esum_all[:, qi:qi + 1],
                    )
            # reduce the 4 quarters per row-tile to a single f32 row sum on
            # the ACT engine (in-order wrt the accum writes above, so no
            # extra semaphores)
            junk = small.tile([128, NT * NQ], f32)
            for t in range(NT):
                nc.scalar.activation(
                    junk[:, t * NQ:(t + 1) * NQ],
                    esum_all[:, t * NQ:(t + 1) * NQ],
                    mybir.ActivationFunctionType.Copy,
                    accum_out=esums_s[:, t:t + 1],
                )
            # single-writer funnel: gives the out DMA exactly one producer
            out_s = small.tile([128, OUTW], f32)
            nc.scalar.activation(out_s[:, :], esums_s[:, :],
                                 mybir.ActivationFunctionType.Copy)
            last_act = nc.cur_bb.bb.instructions[-1]

            nc.sync.drain()
            add_dep_helper(nc.cur_bb.bb.instructions[-1], last_mm, sync=True,
                           reason="observe PE on SP")
            nc.sync.drain()
            add_dep_helper(nc.cur_bb.bb.instructions[-1], last_act, sync=True,
                           reason="observe ACT on SP")
            # issue the out DMA from the ACT engine: it directly follows the
            # ACT Copy in the same stream, so it needs no sync waits at all
            nc.scalar.dma_start(out=outall.ap()[:, :], in_=out_s[:, :])
            out_dma = nc.cur_bb.bb.instructions[-1]
            nc.sync.drain()
            add_dep_helper(nc.cur_bb.bb.instructions[-1], out_dma, sync=True,
                           reason="observe out DMA queue on SP")
    return nc


def _get_runner():
    """Build the Bass module once and return a cached jitted SPMD callable.

    Replicates concourse.bass2jax.run_bass_via_pjrt but keeps the jitted
    function across calls: re-tracing + re-lowering per call costs hundreds
    of ms through the axon tunnel, far more than this kernel's transfers.
    """
    if "runner" in _CACHE:
        return _CACHE["runner"]

    from concourse.bass2jax import (
        _bass_exec_p, install_neuronx_cc_hook, partition_id_tensor,
    )

    nc = _build_bass()
    install_neuronx_cc_hook()

    partition_name = (nc.partition_id_tensor.name
                      if nc.partition_id_tensor else None)
    in_names, out_names, out_avals, zero_shapes = [], [], [], []
    for alloc in nc.m.functions[0].allocations:
        if not isinstance(alloc, mybir.MemoryLocationSet):
            continue
        name = alloc.memorylocations[0].name
        if alloc.kind == "ExternalInput":
            if name != partition_name:
                in_names.append(name)
        elif alloc.kind == "ExternalOutput":
            shape = tuple(alloc.tensor_shape)
            dtype = mybir.dt.np(alloc.dtype)
            out_names.append(name)
            out_avals.append(jax.core.ShapedArray(shape, dtype))
            zero_shapes.append((shape, dtype))
    n_params = len(in_names)
    n_outs = len(out_names)
    in_names_all = list(in_names) + list(out_names)
    if partition_name is not None:
        in_names_all.append(partition_name)
    donate = tuple(range(n_params, n_params + n_outs))

    def _body(*args):
        operands = list(args)
        if partition_name is not None:
            operands.append(partition_id_tensor())
        outs = _bass_exec_p.bind(
            *operands,
            out_avals=tuple(out_avals),
            in_names=tuple(in_names_all),
            out_names=tuple(out_names),
            lowering_input_output_aliases=(),
            sim_require_finite=True,
            sim_require_nnan=True,
            nc=nc,
        )
        return tuple(outs)

    devices = jax.devices()[:NCORES]
    assert len(devices) == NCORES, (
        f"need {NCORES} devices, found {len(jax.devices())}")
    mesh = Mesh(np.asarray(devices), ("core",))
    sharded = jax.jit(
        shard_map(_body, mesh=mesh,
                  in_specs=(PartitionSpec("core"),) * (n_params + n_outs),
                  out_specs=(PartitionSpec("core"),) * n_outs,
                  check_rep=False),
        keep_unused=True,
    )
    # The "output" operands only exist because run_neff-style kernels may
    # rely on pre-zeroed output buffers; this kernel writes every element
    # and the custom call produces fresh result buffers (no aliasing), so
    # park the zeros on device once and never re-upload them.
    from jax.sharding import NamedSharding
    zsh = NamedSharding(mesh, PartitionSpec("core"))
    dev_zeros = [
        jax.device_put(np.zeros((NCORES * s[0], *s[1:]), dt), zsh)
        for s, dt in zero_shapes
    ]
    jax.block_until_ready(dev_zeros)
    _CACHE["runner"] = (sharded, in_names, dev_zeros)
    return _CACHE["runner"]


def kernel(embeddings, labels):
    emb = np.asarray(embeddings, dtype=np.float32)
    labels = np.asarray(labels).astype(np.int64)
    assert emb.shape == (B, D) and labels.shape == (B,)

    sharded, in_names, dev_zeros = _get_runner()

    f8 = mybir.dt.np(mybir.dt.float8e4)
    # eshard_cat[c*128 + d, j] = fp8(emb[c*1024 + j, d]) in one strided pass
    eshard_cat = (emb.reshape(NCORES, RPC, D).transpose(0, 2, 1)
                  .astype(f8).reshape(NCORES * 128, RPC))

    # async dispatch: upload + exec + (eager) fetch run while the host
    # computes the exact q/pc terms below
    out_arrs = sharded(eshard_cat, *dev_zeros)

    emb64 = emb.astype(np.float64)
    nclass = int(labels.max()) + 1
    cnt = np.bincount(labels, minlength=nclass)
    pc = cnt[labels] - 1                      # positives per row (excl. self)
    G = np.zeros((nclass, D), dtype=np.float64)
    np.add.at(G, labels, emb64)
    # q_i = sum over positives j (same label, j != i) of sim[i, j]
    q = (np.einsum("ij,ij->i", emb64, G[labels])
         - np.einsum("ij,ij->i", emb64, emb64)) / TEMP
    has = pc > 0

    oa = np.asarray(out_arrs[0]).reshape(NCORES, 128, OUTW).astype(np.float64)
    # outall[c, p, t] -> esum row sum; outall[c, p, 8+t] -> diag exp.
    # local row j = 128t + p, global row = 1024c + j.
    esum = oa[:, :, :NT].transpose(0, 2, 1).reshape(-1)       # [B]
    dexp = oa[:, :, NT:].transpose(0, 2, 1).reshape(-1)       # [B]
    lse = np.log(esum - dexp)

    row_mean = np.where(has, q / np.maximum(pc, 1) - lse, 0.0)
    loss = -row_mean.sum() / max(int(has.sum()), 1)
    return np.float32(loss)
